# revision 17
# baseline (speedup 1.0000x reference)
"""Trainium2 Bass kernel for a continuous bilinear Koopman operator rollout.

Problem (hardcoded shapes): z0 [256, 256] f32, kernel [256, 256] f32,
log_dt scalar, T=512.  Output: [256, 512, 256] f32 with
out[:, t, :] = z0 @ K_discrete^(t+1),
K_discrete = (I - 0.5*dt*K)^-1 (I + 0.5*dt*K), dt = exp(log_dt).

Strategy:
  - Host computes K_discrete (small [d,d] solve, as the sharding hint
    suggests) and a handful of its powers: A^1..A^16 and A^(16*2^i).
  - z0 and the [B, T, D] output are sharded across 8 cores along batch
    (32 trajectories per core) -- pure data parallelism.
  - On device, the T=512 serial recurrence is restructured as:
      phase B: chunk-start states sT_k = (z0 @ A^(16k)).T for k=0..31,
               built in 5 doubling rounds (s_{k+m} = s_k @ A^(16m)).
      phase C: out rows for chunk k, step j:  s_k @ A^j, j=1..16,
               as matmuls with M=128 (4 chunks x 32 batch) and N=512
               (two consecutive powers) -> PSUM -> SBUF -> 16KB-contiguous
               DMA into out[b, t, :].
  - float32r matmuls: fp32 bits in memory, single-pass PE (1 cycle/row at
    N>=256) instead of float32's LOW/HIGH double pass (4 cycles/row).
"""

import numpy as np

B = 256
D = 256
T = 512
N_CORES = 8
B_LOC = B // N_CORES      # 32
C = 16                    # chunk length (powers A^1..A^C shipped)
N_CHUNKS = T // C         # 32
N_GROUPS = N_CHUNKS // 4  # 8 groups of 4 chunks -> M=128
JP = C // 2               # 8 pairs of consecutive powers -> N=512

_CACHE = {}


def _build_bass():
    import concourse.tile as tile
    from concourse import bacc, mybir

    f32 = mybir.dt.float32
    f32r = mybir.dt.float32r
    nc = bacc.Bacc("TRN2", target_bir_lowering=False, debug=False)

    z0t = nc.dram_tensor("z0t", [D, B_LOC], f32r, kind="ExternalInput").ap()
    # prhs[h, :, (j-1)*256 : j*256] = A^j[h*128:(h+1)*128, :]   j=1..16
    prhs = nc.dram_tensor("prhs", [2, 128, C * D], f32r, kind="ExternalInput").ap()
    # qpow[:, i*512 + h*256 + c] = A^(16*2^i)[h*128 + r, c]     i=0..4
    qpow = nc.dram_tensor("qpow", [128, 5 * 2 * D], f32r, kind="ExternalInput").ap()
    out = nc.dram_tensor("out", [B_LOC, T, D], f32, kind="ExternalOutput").ap()
    # out_r[k, b, j*256 + d] = out[b, 16k + j, d]
    out_r = out.rearrange("b (k j) d -> k b (j d)", j=C)

    with tile.TileContext(nc) as tc:
        with (
            tc.tile_pool(name="const", bufs=1) as cpool,
            tc.tile_pool(name="psum", bufs=8, space="PSUM") as psum_pool,
            tc.tile_pool(name="stage", bufs=6) as stage_pool,
        ):
            # Persistent SBUF tiles.
            # S[h][:, k*32 + b] = s_k[b, h*128 + d']  (chunk starts, transposed)
            S = [
                cpool.tile([128, N_CHUNKS * B_LOC], f32r, name=f"s{h}")
                for h in range(2)
            ]
            P = [cpool.tile([128, C * D], f32r, name=f"p{h}") for h in range(2)]
            Q = cpool.tile([128, 5 * 2 * D], f32r, name="q")

            # Input loads. The SDMA engines round-robin between the two HWDGE
            # queues per packet, so both queues stream concurrently at ~half
            # rate each.  Interleave so each consumer's slice lands just
            # before it is needed: z0t + Q0 first (phase B round 0), then Q
            # rounds alternating with P jp-slices in phase C consumption
            # order.  P's h=1 half streams on the ACT ring, held back behind
            # Q0 by an explicit dep so the tiny phase-B loads are not stuck
            # behind 2MB of P packets.
            from concourse.tile import add_dep_helper

            for h in range(2):
                nc.sync.dma_start(S[h][:, 0:B_LOC], z0t[h * 128:(h + 1) * 128, :])
            q_dmas = []
            for i in range(3):
                q_dmas.append(
                    nc.sync.dma_start(
                        Q[:, i * 512:(i + 1) * 512], qpow[:, i * 512:(i + 1) * 512]
                    )
                )
            nc.sync.dma_start(P[0][:, 0:1024], prhs[0, :, 0:1024])
            q_dmas.append(nc.sync.dma_start(Q[:, 1536:2048], qpow[:, 1536:2048]))
            nc.sync.dma_start(P[0][:, 1024:2048], prhs[0, :, 1024:2048])
            q_dmas.append(nc.sync.dma_start(Q[:, 2048:2560], qpow[:, 2048:2560]))
            nc.sync.dma_start(P[0][:, 2048:3072], prhs[0, :, 2048:3072])
            nc.sync.dma_start(P[0][:, 3072:4096], prhs[0, :, 3072:4096])
            # P h=1 streams on the ACT ring; hold every quarter behind Q0 so
            # the scheduler cannot float any of them ahead of the small
            # phase-B loads (the SDMA engines round-robin between queues).
            for quarter in range(4):
                sl = slice(quarter * 1024, (quarter + 1) * 1024)
                p1 = nc.scalar.dma_start(P[1][:, sl], prhs[1, :, sl])
                add_dep_helper(p1.ins, q_dmas[0].ins, reason="hold P behind Q0")

            # Phase B: doubling rounds. Round i: for k in [0, m),
            #   sT_{k+m} = (A^(16m)).T @ sT_k,  m = 2^i.
            for i in range(5):
                m = 1 << i
                n = B_LOC * m
                for ho in range(2):
                    ps = psum_pool.tile([128, 512], f32, name="psb", tag="ps")
                    for h in range(2):
                        nc.tensor.matmul(
                            ps[:, 0:n],
                            Q[:, i * 512 + h * D + ho * 128:
                               i * 512 + h * D + (ho + 1) * 128],
                            S[h][:, 0:n],
                            start=(h == 0),
                            stop=(h == 1),
                        )
                    nc.vector.tensor_copy(S[ho][:, n:2 * n], ps[:, 0:n])

            # Phase C: group g covers chunks 4g..4g+3 (M = 4 chunks x 32
            # batch = 128 rows).  Split into two halves of 4 jp's each so
            # the stationary operand S[h]-block is reloaded only once per
            # 4 matmuls, and PSUM pressure stays at 4 banks per half.
            # Two half-passes: jp 0..3 for every group first (needs only the
            # low half of P), then jp 4..7.  The first half of the output
            # drains to HBM while P's upper half is still streaming in.
            for half in range(2):
                for g in range(N_GROUPS):
                    stage = stage_pool.tile([128, (C // 2) * D], f32, name="stage")
                    for q in range(4):
                        jp = half * 4 + q
                        ps = psum_pool.tile([128, 512], f32, name="psc", tag="ps")
                        for h in range(2):
                            nc.tensor.matmul(
                                ps[:],
                                S[h][:, g * 128:(g + 1) * 128],
                                P[h][:, jp * 512:(jp + 1) * 512],
                                start=(h == 0),
                                stop=(h == 1),
                            )
                        dst = stage[:, q * 512:(q + 1) * 512]
                        if q == 1:
                            nc.scalar.copy(dst, ps[:])
                        else:
                            nc.vector.tensor_copy(dst, ps[:])
                    # Drain: one DMA per chunk ki ([32, 8, 256] view, outer
                    # dim 32 so HWDGE spreads descriptors over the SDMA
                    # engines; 8KB contiguous per partition).  ki 0/1 (SBUF
                    # partitions 0..63, even AXI ports) on the SP ring,
                    # ki 2/3 (partitions 64..127, odd ports) on the ACT
                    # ring, so in-flight DMAs use complementary SBUF ports.
                    for ki in range(4):
                        k = 4 * g + ki
                        t0c = C * k + half * (C // 2)
                        dma_eng = nc.sync if ki < 2 else nc.scalar
                        dma_eng.dma_start(
                            out[:, t0c: t0c + C // 2, :],
                            stage[ki * B_LOC:(ki + 1) * B_LOC, :],
                        )

    nc.compile()
    return nc


def _host_prep(z0, kernel, log_dt):
    """fp64 host math: K_discrete and its needed powers."""
    K = np.asarray(kernel, dtype=np.float64)
    dt = float(np.exp(np.float64(np.asarray(log_dt))))
    eye = np.eye(D, dtype=np.float64)
    A = np.linalg.solve(eye - 0.5 * dt * K, eye + 0.5 * dt * K)

    pows = [None] * (C + 1)  # pows[j] = A^j
    pows[1] = A
    for j in range(2, C + 1):
        pows[j] = pows[j - 1] @ A

    # qs[i] = A^(C * 2^i), i = 0..4
    qs = [pows[C]]
    for _ in range(4):
        qs.append(qs[-1] @ qs[-1])

    # prhs [2, 128, C*D]
    parr = np.stack([pows[j] for j in range(1, C + 1)], axis=0)  # [16, 256, 256]
    prhs = np.ascontiguousarray(
        parr.reshape(C, 2, 128, D).transpose(1, 2, 0, 3).reshape(2, 128, C * D)
    ).astype(np.float32)

    # qpow [128, 5*2*D]: qpow[r, i*512 + h*256 + c] = qs[i][h*128 + r, c]
    qarr = np.stack(qs, axis=0)  # [5, 256, 256]
    qpow = np.ascontiguousarray(
        qarr.reshape(5, 2, 128, D).transpose(2, 0, 1, 3).reshape(128, 5 * 2 * D)
    ).astype(np.float32)

    z0 = np.asarray(z0, dtype=np.float32)
    z0t_shards = [
        np.ascontiguousarray(z0[c * B_LOC:(c + 1) * B_LOC, :].T) for c in range(N_CORES)
    ]
    return z0t_shards, prhs, qpow


def kernel(**inputs):
    from concourse.bass_utils import run_bass_kernel_spmd

    z0 = inputs["z0"]
    kmat = inputs["kernel"]
    log_dt = inputs["log_dt"]
    t_in = int(np.asarray(inputs["T"]))
    assert t_in == T, f"kernel hardcoded for T={T}, got {t_in}"
    assert tuple(np.shape(z0)) == (B, D)

    z0t_shards, prhs, qpow = _host_prep(z0, kmat, log_dt)

    if "nc" not in _CACHE:
        _CACHE["nc"] = _build_bass()
    nc = _CACHE["nc"]

    in_maps = [
        {"z0t": z0t_shards[c], "prhs": prhs, "qpow": qpow} for c in range(N_CORES)
    ]
    res = run_bass_kernel_spmd(nc, in_maps, core_ids=list(range(N_CORES)))
    return np.concatenate([res.results[c]["out"] for c in range(N_CORES)], axis=0)
